# revision 26
# baseline (speedup 1.0000x reference)
"""Trainium2 Bass kernel for nn_BondMessagePassing (D-MPNN style GNN).

Contract: kernel(**inputs) takes FULL unsharded inputs (as produced by the
reference's setup_inputs) and returns the FULL output [400000, 128] float32.

Math: the reference builds edges in exact reverse pairs (edge 2k+1 is the
reverse of edge 2k, rev_edge_index = i^1), which makes dst[rev] == src.
Consequently the two scatter-adds inside every message-passing iteration
cancel exactly (same multiset of h-rows lands at each node with opposite
sign), so m == 0 in exact arithmetic and h stays at relu(h0 + b_h) for the
whole loop. The output reduces to

    h   = relu(relu([x[src], edge_attr] @ W_i + b_i) + b_h)
    m   = scatter_add(h, dst)            # one scatter, by destination node
    out = relu([x, m] @ W_o + b_o)

This identity is verified at runtime from the actual index tensors (rev is a
bijection with dst[rev] == src); if it does not hold, a numpy fallback
reproduces the reference loop exactly.

Sharding: nodes are range-partitioned across the 8 cores (50000 nodes each);
each core receives exactly the edges whose dst lands in its range (sorted by
dst) so the scatter-add is core-local and the output rows are a contiguous
slice -- no collectives. The scatter-add itself is done on the TensorEngine
as a matmul against a one-hot matrix built on-device with an iota/is_equal
compare, over supertiles of 512 nodes (one PSUM bank per supertile).
"""

import ml_dtypes
import numpy as np

# ---- problem constants (hardcoded per contract) ----
N = 400000
E = 400000
XD = 64        # node feature dim
EAD = 16       # edge feature dim
HID = 128      # hidden dim
DEPTH = 5
NCORES = 8
NL = N // NCORES          # nodes per core
SUP = 512                 # nodes per supertile (one PSUM bank of fp32)
NSUP = (NL + SUP - 1) // SUP
NPAD = NSUP * SUP         # padded nodes per core
P = 128                   # partitions / tile rows

F16 = np.float16
F32 = np.float32


def _check_fast_path_ok(src, dst, rev, x, edge_attr, W_i, b_i, W_h, b_h, W_o, b_o):
    """True iff the loop-cancellation identity holds and fp16 is safe."""
    if src.shape != (E,) or dst.shape != (E,):
        return False
    if rev.shape != (E,):
        return False
    # rev must be a bijection on [0, E)
    if rev.min() < 0 or rev.max() >= E:
        return False
    seen = np.zeros(E, dtype=bool)
    seen[rev] = True
    if not seen.all():
        return False
    # the cancellation identity
    if not np.array_equal(dst[rev], src):
        return False
    if src.min() < 0 or src.max() >= N or dst.min() < 0 or dst.max() >= N:
        return False
    # fp16 magnitude safety (values stream through fp16 operands)
    mx = float(np.abs(x).max(initial=0.0))
    mea = float(np.abs(edge_attr).max(initial=0.0))
    mw = max(float(np.abs(W_i).max(initial=0.0)), float(np.abs(W_o).max(initial=0.0)))
    mb = max(float(np.abs(b_i).max(initial=0.0)), float(np.abs(b_h).max(initial=0.0)),
             float(np.abs(b_o).max(initial=0.0)))
    hbound = 81.0 * max(mx, mea, 1.0) * max(mw, 1.0) + mb
    if not np.isfinite(hbound) or hbound > 2.0e4:
        return False
    return True


def _reference_fallback(x, edge_index, edge_attr, rev_edge_index,
                        W_i, b_i, W_h, b_h, W_o, b_o):
    """Exact numpy replication of the reference (used only if the structural
    identity does not hold, which cannot happen for the documented input
    generator)."""
    def san(t):
        return np.nan_to_num(t, nan=0.0, posinf=1000.0, neginf=-1000.0)

    src, dst = edge_index[0], edge_index[1]
    h0 = np.maximum(
        np.concatenate([x[src], edge_attr], axis=1) @ W_i + b_i, 0.0
    ).astype(F32)
    h = h0
    for _ in range(1, DEPTH):
        m = np.zeros_like(h)
        np.add.at(m, dst, h)
        np.add.at(m, src, -h[rev_edge_index])
        m = san(m) @ W_h + b_h
        h = np.maximum(h0 + m, 0.0).astype(F32)
    m_final = np.zeros_like(h)
    np.add.at(m_final, dst, h)
    h_cat = np.concatenate([x, san(m_final)], axis=1)
    out = np.maximum(h_cat @ W_o + b_o, 0.0).astype(F32)
    return san(out)


_PROGRAM_CACHE = {}


def _build_program(KE, apply_bh, has_bi, has_bo):
    """Build the (SPMD-identical) Bass program.

    KE = edge tiles per supertile: tiles 0..3 hold edges of the fixed
    128-node sub-windows (one-hot column range 128*jj..128*jj+127), tiles
    4..KE-1 are full-width overflow tiles. The layout (which edge goes to
    which tile) is data-dependent, but the program structure is not, so one
    program serves all 8 cores (SPMD).
    """
    import concourse.bacc as bacc
    import concourse.mybir as mybir
    import concourse.tile as tile

    nc = bacc.Bacc("TRN2", target_bir_lowering=False, debug=False,
                   num_devices=NCORES)
    dt = mybir.dt
    EPAD = NSUP * KE * P
    KI = 81 if has_bi else 80   # zt rows (ones row only when b_i != 0)
    KO = 65 if has_bo else 64   # xct rows (ones row only when b_o != 0)
    OV = KE - 4
    # tapered chunk schedule: small head/tail chunks shorten the serial
    # startup (first compute waits on first chunk) and the drain tail
    sched = []
    t = 0
    for g in (1, 2, 4):
        sched.append((t, g)); t += g
    while NSUP - t > 7:
        sched.append((t, 7)); t += 7
    for g in (4, 2, 1):
        if NSUP - t >= g:
            sched.append((t, g)); t += g
    if NSUP - t:
        sched.append((t, NSUP - t))

    zt = nc.dram_tensor("zt", [KI, EPAD], dt.float16, kind="ExternalInput")
    xct = nc.dram_tensor("xct", [KO, NPAD], dt.float16, kind="ExternalInput")
    ldrel = nc.dram_tensor("ldrel", [P, NSUP * OV], dt.float32,
                           kind="ExternalInput")
    s4d = nc.dram_tensor("s4d", [P, NSUP * 4 * P], dt.float8e4,
                         kind="ExternalInput")
    iota = nc.dram_tensor("iota", [P, SUP], dt.float16, kind="ExternalInput")
    w_ih = nc.dram_tensor("w_ih", [KI, HID], dt.float16, kind="ExternalInput")
    w_o1 = nc.dram_tensor("w_o1", [KO, HID], dt.float16, kind="ExternalInput")
    w_o2 = nc.dram_tensor("w_o2", [HID, HID], dt.float16, kind="ExternalInput")
    bh_b = nc.dram_tensor("bh_b", [P, HID], dt.float16, kind="ExternalInput")
    # output is produced TRANSPOSED ([hidden, node]); host transposes back
    outT = nc.dram_tensor("outT", [HID, NPAD], dt.float16,
                          kind="ExternalOutput")

    RELU = mybir.ActivationFunctionType.Relu
    EQ = mybir.AluOpType.is_equal
    NW = 4  # number of fixed-window tiles per supertile

    with tile.TileContext(nc) as tc:
        with (
            tc.tile_pool(name="consts", bufs=1) as consts,
            tc.tile_pool(name="ztp", bufs=3) as ztp,
            tc.tile_pool(name="xctp", bufs=3) as xctp,
            tc.tile_pool(name="hp", bufs=4) as hp,
            tc.tile_pool(name="sp", bufs=3) as sp,
            tc.tile_pool(name="sop", bufs=3) as sop,
            tc.tile_pool(name="mp", bufs=4) as mp,
            tc.tile_pool(name="op", bufs=3) as op,
            tc.tile_pool(name="hps", bufs=2, space="PSUM") as hps,
            tc.tile_pool(name="hops", bufs=1, space="PSUM") as hops,
            tc.tile_pool(name="mps", bufs=3, space="PSUM") as mps,
            tc.tile_pool(name="ops", bufs=2, space="PSUM") as ops,
        ):
            w_ih_t = consts.tile([KI, HID], dt.float16)
            nc.sync.dma_start(out=w_ih_t, in_=w_ih[:, :])
            w_o1_t = consts.tile([KO, HID], dt.float16)
            nc.sync.dma_start(out=w_o1_t, in_=w_o1[:, :])
            w_o2_t = consts.tile([HID, HID], dt.float16)
            nc.sync.dma_start(out=w_o2_t, in_=w_o2[:, :])
            iota_t = consts.tile([P, SUP], dt.float16)
            nc.sync.dma_start(out=iota_t, in_=iota[:, :])
            ldrel_t = consts.tile([P, NSUP * OV], dt.float32)
            nc.sync.dma_start(out=ldrel_t, in_=ldrel[:, :])
            if apply_bh:
                bh_t = consts.tile([P, HID], dt.float16)
                nc.sync.dma_start(out=bh_t, in_=bh_b[:, :])

            def relu_h(h_sbuf, h_psum, ntile=NW):
                nc.scalar.activation(h_sbuf, h_psum, RELU)
                if apply_bh:
                    for q in range(ntile):
                        nc.vector.tensor_tensor(
                            h_sbuf[:, q * HID:(q + 1) * HID],
                            h_sbuf[:, q * HID:(q + 1) * HID], bh_t,
                            op=mybir.AluOpType.add)
                    nc.scalar.activation(h_sbuf, h_sbuf, RELU)

            for TT, Gc in sched:
                zt_c = ztp.tile([KI, Gc * KE * P], dt.float16, tag="ztc")
                nc.sync.dma_start(
                    out=zt_c, in_=zt[:, TT * KE * P:(TT + Gc) * KE * P])
                xct_c = xctp.tile([KO, Gc * SUP], dt.float16, tag="xctc")
                nc.sync.dma_start(
                    out=xct_c, in_=xct[:, TT * SUP:(TT + Gc) * SUP])
                s4_c = sp.tile([P, Gc * NW * P], dt.float8e4, tag="s4c")
                nc.sync.dma_start(
                    out=s4_c,
                    in_=s4d[:, TT * NW * P:(TT + Gc) * NW * P])
                o_buf = op.tile([P, Gc * SUP], dt.float16, tag="obuf")

                # --- overflow h0 for the whole chunk, grouped per bank ---
                per = max(1, (SUP // HID) // OV)  # supertiles per psum bank
                h_sbO = hp.tile([P, Gc * OV * HID], dt.float16, tag="hsbo")
                for base in range(0, Gc, per):
                    cnt = min(per, Gc - base)
                    h_pso = hops.tile([P, cnt * OV * HID], mybir.dt.float32,
                                      tag="hpso")
                    for g2 in range(cnt):
                        zto = zt_c[:, (base + g2) * KE * P:]
                        for i in range(OV):
                            nc.tensor.matmul(
                                h_pso[:, (g2 * OV + i) * HID:
                                      (g2 * OV + i + 1) * HID],
                                zto[:, (NW + i) * P:(NW + i + 1) * P],
                                w_ih_t, start=True, stop=True)
                    dst_sl = h_sbO[:, base * OV * HID:(base + cnt) * OV * HID]
                    if apply_bh:
                        relu_h(dst_sl, h_pso, ntile=cnt * OV)
                    else:
                        nc.vector.tensor_scalar_max(dst_sl, h_pso, 0.0)

                for g in range(Gc):
                    T = TT + g
                    ztg = zt_c[:, g * KE * P:(g + 1) * KE * P]
                    xctg = xct_c[:, g * SUP:(g + 1) * SUP]

                    # --- h0: 4 window tiles in one bank ---
                    h_sb = hp.tile([P, NW * HID], dt.float16)
                    h_ps4 = hps.tile([P, NW * HID], mybir.dt.float32)
                    for jj in range(NW):
                        nc.tensor.matmul(h_ps4[:, jj * HID:(jj + 1) * HID],
                                         ztg[:, jj * P:(jj + 1) * P],
                                         w_ih_t, start=True, stop=True)
                    relu_h(h_sb, h_ps4)

                    # --- one-hot scatter matrices (windows prebuilt fp8) ---
                    s4 = s4_c[:, g * NW * P:(g + 1) * NW * P].rearrange(
                        "p (w q) -> p w q", w=NW)
                    s_o = []
                    for oo in range(NW, KE):
                        jo = T * OV + (oo - NW)
                        s_t = sop.tile([P, SUP], dt.float16)
                        nc.vector.tensor_scalar(
                            s_t, iota_t, ldrel_t[:, jo:jo + 1],
                            None, op0=EQ)
                        s_o.append(s_t)

                    # --- scatter-add on the PE: m_T[h, n] += h^T @ S ---
                    # first writer covers the full bank width (start=True),
                    # everything after accumulates (start=False)
                    m_psum = mps.tile([P, SUP], mybir.dt.float32)
                    for i in range(KE - NW):
                        nc.tensor.matmul(m_psum,
                                         h_sbO[:, (g * OV + i) * HID:
                                               (g * OV + i + 1) * HID],
                                         s_o[i], start=(i == 0), stop=False,
                                         skip_group_check=True)
                    for jj in range(NW):
                        nc.tensor.matmul(m_psum[:, jj * P:(jj + 1) * P],
                                         h_sb[:, jj * HID:(jj + 1) * HID],
                                         s4[:, jj, :], start=False,
                                         stop=(jj == NW - 1),
                                         skip_group_check=True)

                    m_t = mp.tile([P, SUP], dt.float16)
                    nc.vector.tensor_copy(m_t, m_psum)

                    # --- out^T[o, v] = relu(W_o1^T @ xct + W_o2^T @ m_T) ---
                    o_psum = ops.tile([P, SUP], mybir.dt.float32)
                    nc.tensor.matmul(o_psum, w_o1_t, xctg,
                                     start=True, stop=False)
                    nc.tensor.matmul(o_psum, w_o2_t, m_t,
                                     start=False, stop=True)
                    nc.scalar.activation(o_buf[:, g * SUP:(g + 1) * SUP],
                                         o_psum, RELU)

                nc.sync.dma_start(out=outT[:, TT * SUP:(TT + Gc) * SUP],
                                  in_=o_buf)

    nc.compile()
    return nc


def kernel(**inputs):
    x = np.ascontiguousarray(np.asarray(inputs["x"]), dtype=F32)
    edge_index = np.asarray(inputs["edge_index"]).astype(np.int64)
    edge_attr = np.ascontiguousarray(np.asarray(inputs["edge_attr"]), dtype=F32)
    rev = np.asarray(inputs["rev_edge_index"]).astype(np.int64)
    W_i = np.asarray(inputs["W_i"], dtype=F32)
    b_i = np.asarray(inputs["b_i"], dtype=F32)
    W_h = np.asarray(inputs["W_h"], dtype=F32)
    b_h = np.asarray(inputs["b_h"], dtype=F32)
    W_o = np.asarray(inputs["W_o"], dtype=F32)
    b_o = np.asarray(inputs["b_o"], dtype=F32)

    src, dst = edge_index[0], edge_index[1]

    if not _check_fast_path_ok(src, dst, rev, x, edge_attr,
                               W_i, b_i, W_h, b_h, W_o, b_o):
        return _reference_fallback(x, edge_index, edge_attr, rev,
                                   W_i, b_i, W_h, b_h, W_o, b_o)

    from concourse.bass_utils import run_bass_kernel_spmd

    # ---- host-side graph partition / sort (indices only) ----
    order = np.argsort(dst, kind="stable")   # global sort by dst
    dst_s = dst[order]
    # per-core contiguous ranges in the sorted edge list
    core_starts = np.searchsorted(dst_s, np.arange(0, N + NL, NL))

    # Edge-to-slot assignment: supertiles of SUP=512 local nodes, split in 4
    # fixed 128-node windows. The first 128 edges of window jj go to edge
    # tile jj (one-hot columns relative to the window); the rest overflow
    # into full-width tiles 4..KE-1.
    NW = 4
    per_core = []
    KE = 5
    for c in range(NCORES):
        e0, e1 = core_starts[c], core_starts[c + 1]
        ne = e1 - e0
        ld = dst_s[e0:e1] - c * NL           # local dst in [0, NL), sorted
        gidx = ld // P                       # 128-node window id (0..391)
        gstarts = np.searchsorted(ld, np.arange(0, NPAD + P, P))
        r = np.arange(ne) - gstarts[gidx]    # rank within window
        wmask = r < P
        T = ld // SUP                        # supertile id
        tstarts = np.searchsorted(ld, np.arange(0, NPAD + SUP, SUP))
        ocum = np.cumsum(~wmask)             # overflow count up to (incl) i
        prefix = np.concatenate(([0], ocum))
        o_rank = (ocum - 1) - prefix[tstarts[T]]
        n_ov = int(np.sum(~wmask))
        max_ov = int(o_rank[~wmask].max()) + 1 if n_ov else 0
        KE = max(KE, NW + max(1, int(np.ceil(max_ov / P))))
        per_core.append((e0, e1, ld, gidx, r, wmask, T, o_rank))

    # Extremely skewed degree distributions would blow up the overflow
    # capacity (KE); the documented generator (uniform dst) keeps KE at 5.
    if KE > 8:
        return _reference_fallback(x, edge_index, edge_attr, rev,
                                   W_i, b_i, W_h, b_h, W_o, b_o)

    apply_bh = bool(np.any(b_h))
    has_bi = bool(np.any(b_i))
    has_bo = bool(np.any(b_o))
    KI = 81 if has_bi else 80
    KO = 65 if has_bo else 64
    OV = KE - NW
    EPAD = NSUP * KE * P

    # ---- shared constant tensors ----
    w_ih_np = (np.concatenate([W_i, b_i[None, :]], axis=0) if has_bi
               else W_i).astype(F16)
    w_o1_np = (np.concatenate([W_o[:XD], b_o[None, :]], axis=0) if has_bo
               else W_o[:XD]).astype(F16)
    w_o2_np = np.ascontiguousarray(W_o[XD:]).astype(F16)                   # [128,128]
    iota_np = np.broadcast_to(np.arange(SUP, dtype=F16), (P, SUP)).copy()
    bh_np = np.broadcast_to(b_h.astype(F16), (P, HID)).copy()

    x16t = np.ascontiguousarray(x.T.astype(F16))            # [64, N]
    ea16t = np.ascontiguousarray(edge_attr.T.astype(F16))   # [16, E]

    in_maps = []
    for c in range(NCORES):
        e0, e1, ld, gidx, r, wmask, T, o_rank = per_core[c]
        eids = order[e0:e1]

        # slot per edge: window edges -> tile (gidx%4) of supertile T at
        # rank r; overflow edges -> tiles NW.. at rank o_rank.
        base = T * (KE * P)
        slots = np.where(
            wmask,
            base + (gidx % NW) * P + r,
            base + NW * P + o_rank,
        )
        # one-hot column value (window tiles are window-relative)
        ldrel_val = np.where(wmask, ld % P, ld % SUP).astype(F32)

        zt_np = np.zeros((KI, EPAD), dtype=F16)
        zt_np[0:XD, slots] = x16t[:, src[eids]]
        zt_np[XD:XD + EAD, slots] = ea16t[:, eids]
        if has_bi:
            zt_np[80, slots] = 1.0

        # overflow-tile one-hot scalar columns only (window tiles use s4d)
        ldrel_np = np.full((P, NSUP * OV), -1.0, dtype=F32)
        ov = ~wmask
        oslot = slots[ov]
        otile = (oslot // P)                 # global edge tile id T*KE + NW + i
        ocol = (otile // KE) * OV + (otile % KE - NW)
        ldrel_np[oslot % P, ocol] = ldrel_val[ov]

        s4_np = np.zeros((P, NSUP * NW * P), dtype=ml_dtypes.float8_e4m3)
        ws = wmask
        s4_np[r[ws], T[ws] * (NW * P) + (gidx[ws] % NW) * P + (ld[ws] % P)] = 1.0

        xct_np = np.zeros((KO, NPAD), dtype=F16)
        xct_np[0:XD, :NL] = x16t[:, c * NL:(c + 1) * NL]
        if has_bo:
            xct_np[64, :] = 1.0

        in_maps.append({
            "zt": zt_np, "xct": xct_np, "ldrel": ldrel_np,
            "s4d": s4_np, "iota": iota_np,
            "w_ih": w_ih_np, "w_o1": w_o1_np, "w_o2": w_o2_np, "bh_b": bh_np,
        })

    key = (KE, apply_bh, has_bi, has_bo)
    if key not in _PROGRAM_CACHE:
        _PROGRAM_CACHE[key] = _build_program(KE, apply_bh, has_bi, has_bo)
    nc = _PROGRAM_CACHE[key]

    import os
    trace = bool(os.environ.get("BMP_TRACE"))
    res = run_bass_kernel_spmd(nc, in_maps, core_ids=list(range(NCORES)),
                               trace=trace)
    if trace:
        global LAST_EXEC_TIME_NS, LAST_TRACE
        LAST_EXEC_TIME_NS = res.exec_time_ns
        LAST_TRACE = res.instructions_and_trace
    out = np.empty((N, HID), dtype=F32)
    for c in range(NCORES):
        out[c * NL:(c + 1) * NL] = res.results[c]["outT"][:, :NL].T.astype(F32)
    return out


# revision 27
# speedup vs baseline: 1.0030x; 1.0030x over previous
"""Trainium2 Bass kernel for nn_BondMessagePassing (D-MPNN style GNN).

Contract: kernel(**inputs) takes FULL unsharded inputs (as produced by the
reference's setup_inputs) and returns the FULL output [400000, 128] float32.

Math: the reference builds edges in exact reverse pairs (edge 2k+1 is the
reverse of edge 2k, rev_edge_index = i^1), which makes dst[rev] == src.
Consequently the two scatter-adds inside every message-passing iteration
cancel exactly (same multiset of h-rows lands at each node with opposite
sign), so m == 0 in exact arithmetic and h stays at relu(h0 + b_h) for the
whole loop. The output reduces to

    h   = relu(relu([x[src], edge_attr] @ W_i + b_i) + b_h)
    m   = scatter_add(h, dst)            # one scatter, by destination node
    out = relu([x, m] @ W_o + b_o)

This identity is verified at runtime from the actual index tensors (rev is a
bijection with dst[rev] == src); if it does not hold, a numpy fallback
reproduces the reference loop exactly.

Sharding: nodes are range-partitioned across the 8 cores (50000 nodes each);
each core receives exactly the edges whose dst lands in its range (sorted by
dst) so the scatter-add is core-local and the output rows are a contiguous
slice -- no collectives. The scatter-add itself is done on the TensorEngine
as a matmul against a one-hot matrix built on-device with an iota/is_equal
compare, over supertiles of 512 nodes (one PSUM bank per supertile).
"""

import ml_dtypes
import numpy as np

# ---- problem constants (hardcoded per contract) ----
N = 400000
E = 400000
XD = 64        # node feature dim
EAD = 16       # edge feature dim
HID = 128      # hidden dim
DEPTH = 5
NCORES = 8
NL = N // NCORES          # nodes per core
SUP = 512                 # nodes per supertile (one PSUM bank of fp32)
NSUP = (NL + SUP - 1) // SUP
NPAD = NSUP * SUP         # padded nodes per core
P = 128                   # partitions / tile rows

F16 = np.float16
F32 = np.float32


def _check_fast_path_ok(src, dst, rev, x, edge_attr, W_i, b_i, W_h, b_h, W_o, b_o):
    """True iff the loop-cancellation identity holds and fp16 is safe."""
    if src.shape != (E,) or dst.shape != (E,):
        return False
    if rev.shape != (E,):
        return False
    # rev must be a bijection on [0, E)
    if rev.min() < 0 or rev.max() >= E:
        return False
    seen = np.zeros(E, dtype=bool)
    seen[rev] = True
    if not seen.all():
        return False
    # the cancellation identity
    if not np.array_equal(dst[rev], src):
        return False
    if src.min() < 0 or src.max() >= N or dst.min() < 0 or dst.max() >= N:
        return False
    # fp16 magnitude safety (values stream through fp16 operands)
    mx = float(np.abs(x).max(initial=0.0))
    mea = float(np.abs(edge_attr).max(initial=0.0))
    mw = max(float(np.abs(W_i).max(initial=0.0)), float(np.abs(W_o).max(initial=0.0)))
    mb = max(float(np.abs(b_i).max(initial=0.0)), float(np.abs(b_h).max(initial=0.0)),
             float(np.abs(b_o).max(initial=0.0)))
    hbound = 81.0 * max(mx, mea, 1.0) * max(mw, 1.0) + mb
    if not np.isfinite(hbound) or hbound > 2.0e4:
        return False
    return True


def _reference_fallback(x, edge_index, edge_attr, rev_edge_index,
                        W_i, b_i, W_h, b_h, W_o, b_o):
    """Exact numpy replication of the reference (used only if the structural
    identity does not hold, which cannot happen for the documented input
    generator)."""
    def san(t):
        return np.nan_to_num(t, nan=0.0, posinf=1000.0, neginf=-1000.0)

    src, dst = edge_index[0], edge_index[1]
    h0 = np.maximum(
        np.concatenate([x[src], edge_attr], axis=1) @ W_i + b_i, 0.0
    ).astype(F32)
    h = h0
    for _ in range(1, DEPTH):
        m = np.zeros_like(h)
        np.add.at(m, dst, h)
        np.add.at(m, src, -h[rev_edge_index])
        m = san(m) @ W_h + b_h
        h = np.maximum(h0 + m, 0.0).astype(F32)
    m_final = np.zeros_like(h)
    np.add.at(m_final, dst, h)
    h_cat = np.concatenate([x, san(m_final)], axis=1)
    out = np.maximum(h_cat @ W_o + b_o, 0.0).astype(F32)
    return san(out)


_PROGRAM_CACHE = {}


def _build_program(KE, apply_bh, has_bi, has_bo):
    """Build the (SPMD-identical) Bass program.

    KE = edge tiles per supertile: tiles 0..3 hold edges of the fixed
    128-node sub-windows (one-hot column range 128*jj..128*jj+127), tiles
    4..KE-1 are full-width overflow tiles. The layout (which edge goes to
    which tile) is data-dependent, but the program structure is not, so one
    program serves all 8 cores (SPMD).
    """
    import concourse.bacc as bacc
    import concourse.mybir as mybir
    import concourse.tile as tile

    nc = bacc.Bacc("TRN2", target_bir_lowering=False, debug=False,
                   num_devices=NCORES)
    dt = mybir.dt
    EPAD = NSUP * KE * P
    KI = 81 if has_bi else 80   # zt rows (ones row only when b_i != 0)
    KO = 65 if has_bo else 64   # xct rows (ones row only when b_o != 0)
    OV = KE - 4
    G = 7  # supertiles per DMA chunk
    sched = [(t, min(G, NSUP - t)) for t in range(0, NSUP, G)]

    zt = nc.dram_tensor("zt", [KI, EPAD], dt.float16, kind="ExternalInput")
    xct = nc.dram_tensor("xct", [KO, NPAD], dt.float16, kind="ExternalInput")
    ldrel = nc.dram_tensor("ldrel", [P, NSUP * OV], dt.float32,
                           kind="ExternalInput")
    s4d = nc.dram_tensor("s4d", [P, NSUP * 4 * P], dt.float8e4,
                         kind="ExternalInput")
    iota = nc.dram_tensor("iota", [P, SUP], dt.float16, kind="ExternalInput")
    w_ih = nc.dram_tensor("w_ih", [KI, HID], dt.float16, kind="ExternalInput")
    w_o1 = nc.dram_tensor("w_o1", [KO, HID], dt.float16, kind="ExternalInput")
    w_o2 = nc.dram_tensor("w_o2", [HID, HID], dt.float16, kind="ExternalInput")
    bh_b = nc.dram_tensor("bh_b", [P, HID], dt.float16, kind="ExternalInput")
    # output is produced TRANSPOSED ([hidden, node]); host transposes back
    outT = nc.dram_tensor("outT", [HID, NPAD], dt.float16,
                          kind="ExternalOutput")

    RELU = mybir.ActivationFunctionType.Relu
    EQ = mybir.AluOpType.is_equal
    NW = 4  # number of fixed-window tiles per supertile

    with tile.TileContext(nc) as tc:
        with (
            tc.tile_pool(name="consts", bufs=1) as consts,
            tc.tile_pool(name="ztp", bufs=3) as ztp,
            tc.tile_pool(name="xctp", bufs=3) as xctp,
            tc.tile_pool(name="hp", bufs=4) as hp,
            tc.tile_pool(name="sp", bufs=3) as sp,
            tc.tile_pool(name="sop", bufs=3) as sop,
            tc.tile_pool(name="mp", bufs=4) as mp,
            tc.tile_pool(name="op", bufs=3) as op,
            tc.tile_pool(name="hps", bufs=2, space="PSUM") as hps,
            tc.tile_pool(name="hops", bufs=1, space="PSUM") as hops,
            tc.tile_pool(name="mps", bufs=3, space="PSUM") as mps,
            tc.tile_pool(name="ops", bufs=2, space="PSUM") as ops,
        ):
            w_ih_t = consts.tile([KI, HID], dt.float16)
            nc.sync.dma_start(out=w_ih_t, in_=w_ih[:, :])
            w_o1_t = consts.tile([KO, HID], dt.float16)
            nc.sync.dma_start(out=w_o1_t, in_=w_o1[:, :])
            w_o2_t = consts.tile([HID, HID], dt.float16)
            nc.sync.dma_start(out=w_o2_t, in_=w_o2[:, :])
            iota_t = consts.tile([P, SUP], dt.float16)
            nc.sync.dma_start(out=iota_t, in_=iota[:, :])
            ldrel_t = consts.tile([P, NSUP * OV], dt.float32)
            nc.sync.dma_start(out=ldrel_t, in_=ldrel[:, :])
            if apply_bh:
                bh_t = consts.tile([P, HID], dt.float16)
                nc.sync.dma_start(out=bh_t, in_=bh_b[:, :])

            def relu_h(h_sbuf, h_psum, ntile=NW):
                nc.scalar.activation(h_sbuf, h_psum, RELU)
                if apply_bh:
                    for q in range(ntile):
                        nc.vector.tensor_tensor(
                            h_sbuf[:, q * HID:(q + 1) * HID],
                            h_sbuf[:, q * HID:(q + 1) * HID], bh_t,
                            op=mybir.AluOpType.add)
                    nc.scalar.activation(h_sbuf, h_sbuf, RELU)

            for TT, Gc in sched:
                zt_c = ztp.tile([KI, Gc * KE * P], dt.float16, tag="ztc")
                nc.sync.dma_start(
                    out=zt_c, in_=zt[:, TT * KE * P:(TT + Gc) * KE * P])
                xct_c = xctp.tile([KO, Gc * SUP], dt.float16, tag="xctc")
                nc.sync.dma_start(
                    out=xct_c, in_=xct[:, TT * SUP:(TT + Gc) * SUP])
                s4_c = sp.tile([P, Gc * NW * P], dt.float8e4, tag="s4c")
                nc.sync.dma_start(
                    out=s4_c,
                    in_=s4d[:, TT * NW * P:(TT + Gc) * NW * P])
                o_buf = op.tile([P, Gc * SUP], dt.float16, tag="obuf")

                # --- overflow h0 for the whole chunk, grouped per bank ---
                per = max(1, (SUP // HID) // OV)  # supertiles per psum bank
                h_sbO = hp.tile([P, Gc * OV * HID], dt.float16, tag="hsbo")
                for base in range(0, Gc, per):
                    cnt = min(per, Gc - base)
                    h_pso = hops.tile([P, cnt * OV * HID], mybir.dt.float32,
                                      tag="hpso")
                    for g2 in range(cnt):
                        zto = zt_c[:, (base + g2) * KE * P:]
                        for i in range(OV):
                            nc.tensor.matmul(
                                h_pso[:, (g2 * OV + i) * HID:
                                      (g2 * OV + i + 1) * HID],
                                zto[:, (NW + i) * P:(NW + i + 1) * P],
                                w_ih_t, start=True, stop=True)
                    dst_sl = h_sbO[:, base * OV * HID:(base + cnt) * OV * HID]
                    if apply_bh:
                        relu_h(dst_sl, h_pso, ntile=cnt * OV)
                    else:
                        nc.vector.tensor_scalar_max(dst_sl, h_pso, 0.0)

                for g in range(Gc):
                    T = TT + g
                    ztg = zt_c[:, g * KE * P:(g + 1) * KE * P]
                    xctg = xct_c[:, g * SUP:(g + 1) * SUP]

                    # --- h0: 4 window tiles in one bank ---
                    h_sb = hp.tile([P, NW * HID], dt.float16)
                    h_ps4 = hps.tile([P, NW * HID], mybir.dt.float32)
                    for jj in range(NW):
                        nc.tensor.matmul(h_ps4[:, jj * HID:(jj + 1) * HID],
                                         ztg[:, jj * P:(jj + 1) * P],
                                         w_ih_t, start=True, stop=True)
                    relu_h(h_sb, h_ps4)

                    # --- one-hot scatter matrices (windows prebuilt fp8) ---
                    s4 = s4_c[:, g * NW * P:(g + 1) * NW * P].rearrange(
                        "p (w q) -> p w q", w=NW)
                    s_o = []
                    for oo in range(NW, KE):
                        jo = T * OV + (oo - NW)
                        s_t = sop.tile([P, SUP], dt.float16)
                        nc.vector.tensor_scalar(
                            s_t, iota_t, ldrel_t[:, jo:jo + 1],
                            None, op0=EQ)
                        s_o.append(s_t)

                    # --- scatter-add on the PE: m_T[h, n] += h^T @ S ---
                    # first writer covers the full bank width (start=True),
                    # everything after accumulates (start=False)
                    m_psum = mps.tile([P, SUP], mybir.dt.float32)
                    for i in range(KE - NW):
                        nc.tensor.matmul(m_psum,
                                         h_sbO[:, (g * OV + i) * HID:
                                               (g * OV + i + 1) * HID],
                                         s_o[i], start=(i == 0), stop=False,
                                         skip_group_check=True)
                    for jj in range(NW):
                        nc.tensor.matmul(m_psum[:, jj * P:(jj + 1) * P],
                                         h_sb[:, jj * HID:(jj + 1) * HID],
                                         s4[:, jj, :], start=False,
                                         stop=(jj == NW - 1),
                                         skip_group_check=True)

                    m_t = mp.tile([P, SUP], dt.float16)
                    nc.vector.tensor_copy(m_t, m_psum)

                    # --- out^T[o, v] = relu(W_o1^T @ xct + W_o2^T @ m_T) ---
                    o_psum = ops.tile([P, SUP], mybir.dt.float32)
                    nc.tensor.matmul(o_psum, w_o1_t, xctg,
                                     start=True, stop=False)
                    nc.tensor.matmul(o_psum, w_o2_t, m_t,
                                     start=False, stop=True)
                    nc.scalar.activation(o_buf[:, g * SUP:(g + 1) * SUP],
                                         o_psum, RELU)

                nc.sync.dma_start(out=outT[:, TT * SUP:(TT + Gc) * SUP],
                                  in_=o_buf)

    nc.compile()
    return nc


def kernel(**inputs):
    x = np.ascontiguousarray(np.asarray(inputs["x"]), dtype=F32)
    edge_index = np.asarray(inputs["edge_index"]).astype(np.int64)
    edge_attr = np.ascontiguousarray(np.asarray(inputs["edge_attr"]), dtype=F32)
    rev = np.asarray(inputs["rev_edge_index"]).astype(np.int64)
    W_i = np.asarray(inputs["W_i"], dtype=F32)
    b_i = np.asarray(inputs["b_i"], dtype=F32)
    W_h = np.asarray(inputs["W_h"], dtype=F32)
    b_h = np.asarray(inputs["b_h"], dtype=F32)
    W_o = np.asarray(inputs["W_o"], dtype=F32)
    b_o = np.asarray(inputs["b_o"], dtype=F32)

    src, dst = edge_index[0], edge_index[1]

    if not _check_fast_path_ok(src, dst, rev, x, edge_attr,
                               W_i, b_i, W_h, b_h, W_o, b_o):
        return _reference_fallback(x, edge_index, edge_attr, rev,
                                   W_i, b_i, W_h, b_h, W_o, b_o)

    from concourse.bass_utils import run_bass_kernel_spmd

    # ---- host-side graph partition / sort (indices only) ----
    order = np.argsort(dst, kind="stable")   # global sort by dst
    dst_s = dst[order]
    # per-core contiguous ranges in the sorted edge list
    core_starts = np.searchsorted(dst_s, np.arange(0, N + NL, NL))

    # Edge-to-slot assignment: supertiles of SUP=512 local nodes, split in 4
    # fixed 128-node windows. The first 128 edges of window jj go to edge
    # tile jj (one-hot columns relative to the window); the rest overflow
    # into full-width tiles 4..KE-1.
    NW = 4
    per_core = []
    KE = 5
    for c in range(NCORES):
        e0, e1 = core_starts[c], core_starts[c + 1]
        ne = e1 - e0
        ld = dst_s[e0:e1] - c * NL           # local dst in [0, NL), sorted
        gidx = ld // P                       # 128-node window id (0..391)
        gstarts = np.searchsorted(ld, np.arange(0, NPAD + P, P))
        r = np.arange(ne) - gstarts[gidx]    # rank within window
        wmask = r < P
        T = ld // SUP                        # supertile id
        tstarts = np.searchsorted(ld, np.arange(0, NPAD + SUP, SUP))
        ocum = np.cumsum(~wmask)             # overflow count up to (incl) i
        prefix = np.concatenate(([0], ocum))
        o_rank = (ocum - 1) - prefix[tstarts[T]]
        n_ov = int(np.sum(~wmask))
        max_ov = int(o_rank[~wmask].max()) + 1 if n_ov else 0
        KE = max(KE, NW + max(1, int(np.ceil(max_ov / P))))
        per_core.append((e0, e1, ld, gidx, r, wmask, T, o_rank))

    # Extremely skewed degree distributions would blow up the overflow
    # capacity (KE); the documented generator (uniform dst) keeps KE at 5.
    if KE > 8:
        return _reference_fallback(x, edge_index, edge_attr, rev,
                                   W_i, b_i, W_h, b_h, W_o, b_o)

    apply_bh = bool(np.any(b_h))
    has_bi = bool(np.any(b_i))
    has_bo = bool(np.any(b_o))
    KI = 81 if has_bi else 80
    KO = 65 if has_bo else 64
    OV = KE - NW
    EPAD = NSUP * KE * P

    # ---- shared constant tensors ----
    w_ih_np = (np.concatenate([W_i, b_i[None, :]], axis=0) if has_bi
               else W_i).astype(F16)
    w_o1_np = (np.concatenate([W_o[:XD], b_o[None, :]], axis=0) if has_bo
               else W_o[:XD]).astype(F16)
    w_o2_np = np.ascontiguousarray(W_o[XD:]).astype(F16)                   # [128,128]
    iota_np = np.broadcast_to(np.arange(SUP, dtype=F16), (P, SUP)).copy()
    bh_np = np.broadcast_to(b_h.astype(F16), (P, HID)).copy()

    x16t = np.ascontiguousarray(x.T.astype(F16))            # [64, N]
    ea16t = np.ascontiguousarray(edge_attr.T.astype(F16))   # [16, E]

    in_maps = []
    for c in range(NCORES):
        e0, e1, ld, gidx, r, wmask, T, o_rank = per_core[c]
        eids = order[e0:e1]

        # slot per edge: window edges -> tile (gidx%4) of supertile T at
        # rank r; overflow edges -> tiles NW.. at rank o_rank.
        base = T * (KE * P)
        slots = np.where(
            wmask,
            base + (gidx % NW) * P + r,
            base + NW * P + o_rank,
        )
        # one-hot column value (window tiles are window-relative)
        ldrel_val = np.where(wmask, ld % P, ld % SUP).astype(F32)

        zt_np = np.zeros((KI, EPAD), dtype=F16)
        zt_np[0:XD, slots] = x16t[:, src[eids]]
        zt_np[XD:XD + EAD, slots] = ea16t[:, eids]
        if has_bi:
            zt_np[80, slots] = 1.0

        # overflow-tile one-hot scalar columns only (window tiles use s4d)
        ldrel_np = np.full((P, NSUP * OV), -1.0, dtype=F32)
        ov = ~wmask
        oslot = slots[ov]
        otile = (oslot // P)                 # global edge tile id T*KE + NW + i
        ocol = (otile // KE) * OV + (otile % KE - NW)
        ldrel_np[oslot % P, ocol] = ldrel_val[ov]

        s4_np = np.zeros((P, NSUP * NW * P), dtype=ml_dtypes.float8_e4m3)
        ws = wmask
        s4_np[r[ws], T[ws] * (NW * P) + (gidx[ws] % NW) * P + (ld[ws] % P)] = 1.0

        xct_np = np.zeros((KO, NPAD), dtype=F16)
        xct_np[0:XD, :NL] = x16t[:, c * NL:(c + 1) * NL]
        if has_bo:
            xct_np[64, :] = 1.0

        in_maps.append({
            "zt": zt_np, "xct": xct_np, "ldrel": ldrel_np,
            "s4d": s4_np, "iota": iota_np,
            "w_ih": w_ih_np, "w_o1": w_o1_np, "w_o2": w_o2_np, "bh_b": bh_np,
        })

    key = (KE, apply_bh, has_bi, has_bo)
    if key not in _PROGRAM_CACHE:
        _PROGRAM_CACHE[key] = _build_program(KE, apply_bh, has_bi, has_bo)
    nc = _PROGRAM_CACHE[key]

    import os
    trace = bool(os.environ.get("BMP_TRACE"))
    res = run_bass_kernel_spmd(nc, in_maps, core_ids=list(range(NCORES)),
                               trace=trace)
    if trace:
        global LAST_EXEC_TIME_NS, LAST_TRACE
        LAST_EXEC_TIME_NS = res.exec_time_ns
        LAST_TRACE = res.instructions_and_trace
    out = np.empty((N, HID), dtype=F32)
    for c in range(NCORES):
        out[c * NL:(c + 1) * NL] = res.results[c]["outT"][:, :NL].T.astype(F32)
    return out


# revision 28
# speedup vs baseline: 1.0112x; 1.0081x over previous
"""Trainium2 Bass kernel for nn_BondMessagePassing (D-MPNN style GNN).

Contract: kernel(**inputs) takes FULL unsharded inputs (as produced by the
reference's setup_inputs) and returns the FULL output [400000, 128] float32.

Math: the reference builds edges in exact reverse pairs (edge 2k+1 is the
reverse of edge 2k, rev_edge_index = i^1), which makes dst[rev] == src.
Consequently the two scatter-adds inside every message-passing iteration
cancel exactly (same multiset of h-rows lands at each node with opposite
sign), so m == 0 in exact arithmetic and h stays at relu(h0 + b_h) for the
whole loop. The output reduces to

    h   = relu(relu([x[src], edge_attr] @ W_i + b_i) + b_h)
    m   = scatter_add(h, dst)            # one scatter, by destination node
    out = relu([x, m] @ W_o + b_o)

This identity is verified at runtime from the actual index tensors (rev is a
bijection with dst[rev] == src); if it does not hold, a numpy fallback
reproduces the reference loop exactly.

Sharding: nodes are range-partitioned across the 8 cores (50000 nodes each);
each core receives exactly the edges whose dst lands in its range (sorted by
dst) so the scatter-add is core-local and the output rows are a contiguous
slice -- no collectives. The scatter-add itself is done on the TensorEngine
as a matmul against a one-hot matrix built on-device with an iota/is_equal
compare, over supertiles of 512 nodes (one PSUM bank per supertile).
"""

import ml_dtypes
import numpy as np

# ---- problem constants (hardcoded per contract) ----
N = 400000
E = 400000
XD = 64        # node feature dim
EAD = 16       # edge feature dim
HID = 128      # hidden dim
DEPTH = 5
NCORES = 8
NL = N // NCORES          # nodes per core
SUP = 512                 # nodes per supertile (one PSUM bank of fp32)
NSUP = (NL + SUP - 1) // SUP
NPAD = NSUP * SUP         # padded nodes per core
P = 128                   # partitions / tile rows

F16 = np.float16
F32 = np.float32


def _check_fast_path_ok(src, dst, rev, x, edge_attr, W_i, b_i, W_h, b_h, W_o, b_o):
    """True iff the loop-cancellation identity holds and fp16 is safe."""
    if src.shape != (E,) or dst.shape != (E,):
        return False
    if rev.shape != (E,):
        return False
    # rev must be a bijection on [0, E)
    if rev.min() < 0 or rev.max() >= E:
        return False
    seen = np.zeros(E, dtype=bool)
    seen[rev] = True
    if not seen.all():
        return False
    # the cancellation identity
    if not np.array_equal(dst[rev], src):
        return False
    if src.min() < 0 or src.max() >= N or dst.min() < 0 or dst.max() >= N:
        return False
    # fp16 magnitude safety (values stream through fp16 operands)
    mx = float(np.abs(x).max(initial=0.0))
    mea = float(np.abs(edge_attr).max(initial=0.0))
    mw = max(float(np.abs(W_i).max(initial=0.0)), float(np.abs(W_o).max(initial=0.0)))
    mb = max(float(np.abs(b_i).max(initial=0.0)), float(np.abs(b_h).max(initial=0.0)),
             float(np.abs(b_o).max(initial=0.0)))
    hbound = 81.0 * max(mx, mea, 1.0) * max(mw, 1.0) + mb
    if not np.isfinite(hbound) or hbound > 2.0e4:
        return False
    return True


def _reference_fallback(x, edge_index, edge_attr, rev_edge_index,
                        W_i, b_i, W_h, b_h, W_o, b_o):
    """Exact numpy replication of the reference (used only if the structural
    identity does not hold, which cannot happen for the documented input
    generator)."""
    def san(t):
        return np.nan_to_num(t, nan=0.0, posinf=1000.0, neginf=-1000.0)

    src, dst = edge_index[0], edge_index[1]
    h0 = np.maximum(
        np.concatenate([x[src], edge_attr], axis=1) @ W_i + b_i, 0.0
    ).astype(F32)
    h = h0
    for _ in range(1, DEPTH):
        m = np.zeros_like(h)
        np.add.at(m, dst, h)
        np.add.at(m, src, -h[rev_edge_index])
        m = san(m) @ W_h + b_h
        h = np.maximum(h0 + m, 0.0).astype(F32)
    m_final = np.zeros_like(h)
    np.add.at(m_final, dst, h)
    h_cat = np.concatenate([x, san(m_final)], axis=1)
    out = np.maximum(h_cat @ W_o + b_o, 0.0).astype(F32)
    return san(out)


_PROGRAM_CACHE = {}


def _build_program(KE, apply_bh):
    """Build the (SPMD-identical) Bass program.

    KE = edge tiles per supertile: tiles 0..3 hold edges of the fixed
    128-node sub-windows (one-hot column range 128*jj..128*jj+127), tiles
    4..KE-1 are full-width overflow tiles. The layout (which edge goes to
    which tile) is data-dependent, but the program structure is not, so one
    program serves all 8 cores (SPMD).
    """
    import concourse.bacc as bacc
    import concourse.mybir as mybir
    import concourse.tile as tile

    nc = bacc.Bacc("TRN2", target_bir_lowering=False, debug=False,
                   num_devices=NCORES)
    dt = mybir.dt
    EPAD = NSUP * KE * P
    G = 7  # supertiles per DMA chunk

    zt = nc.dram_tensor("zt", [81, EPAD], dt.float16, kind="ExternalInput")
    xct = nc.dram_tensor("xct", [65, NPAD], dt.float16, kind="ExternalInput")
    ldrel = nc.dram_tensor("ldrel", [P, NSUP * KE], dt.float32,
                           kind="ExternalInput")
    s4d = nc.dram_tensor("s4d", [P, NSUP * 4 * P], dt.float8e4,
                         kind="ExternalInput")
    iota = nc.dram_tensor("iota", [P, SUP], dt.float16, kind="ExternalInput")
    w_ih = nc.dram_tensor("w_ih", [81, HID], dt.float16, kind="ExternalInput")
    w_o1 = nc.dram_tensor("w_o1", [65, HID], dt.float16, kind="ExternalInput")
    w_o2 = nc.dram_tensor("w_o2", [HID, HID], dt.float16, kind="ExternalInput")
    bh_b = nc.dram_tensor("bh_b", [P, HID], dt.float16, kind="ExternalInput")
    # output is produced TRANSPOSED ([hidden, node]); host transposes back
    outT = nc.dram_tensor("outT", [HID, NPAD], dt.float16,
                          kind="ExternalOutput")

    RELU = mybir.ActivationFunctionType.Relu
    EQ = mybir.AluOpType.is_equal
    NW = 4  # number of fixed-window tiles per supertile

    with tile.TileContext(nc) as tc:
        with (
            tc.tile_pool(name="consts", bufs=1) as consts,
            tc.tile_pool(name="ztp", bufs=3) as ztp,
            tc.tile_pool(name="xctp", bufs=3) as xctp,
            tc.tile_pool(name="hp", bufs=4) as hp,
            tc.tile_pool(name="sp", bufs=3) as sp,
            tc.tile_pool(name="sop", bufs=3) as sop,
            tc.tile_pool(name="mp", bufs=4) as mp,
            tc.tile_pool(name="op", bufs=3) as op,
            tc.tile_pool(name="hps", bufs=2, space="PSUM") as hps,
            tc.tile_pool(name="hops", bufs=1, space="PSUM") as hops,
            tc.tile_pool(name="mps", bufs=3, space="PSUM") as mps,
            tc.tile_pool(name="ops", bufs=2, space="PSUM") as ops,
        ):
            w_ih_t = consts.tile([81, HID], dt.float16)
            nc.sync.dma_start(out=w_ih_t, in_=w_ih[:, :])
            w_o1_t = consts.tile([65, HID], dt.float16)
            nc.sync.dma_start(out=w_o1_t, in_=w_o1[:, :])
            w_o2_t = consts.tile([HID, HID], dt.float16)
            nc.sync.dma_start(out=w_o2_t, in_=w_o2[:, :])
            iota_t = consts.tile([P, SUP], dt.float16)
            nc.sync.dma_start(out=iota_t, in_=iota[:, :])
            ldrel_t = consts.tile([P, NSUP * KE], dt.float32)
            nc.sync.dma_start(out=ldrel_t, in_=ldrel[:, :])
            if apply_bh:
                bh_t = consts.tile([P, HID], dt.float16)
                nc.sync.dma_start(out=bh_t, in_=bh_b[:, :])

            def relu_h(h_sbuf, h_psum, ntile=NW):
                nc.scalar.activation(h_sbuf, h_psum, RELU)
                if apply_bh:
                    for q in range(ntile):
                        nc.vector.tensor_tensor(
                            h_sbuf[:, q * HID:(q + 1) * HID],
                            h_sbuf[:, q * HID:(q + 1) * HID], bh_t,
                            op=mybir.AluOpType.add)
                    nc.scalar.activation(h_sbuf, h_sbuf, RELU)

            for TT in range(0, NSUP, G):
                Gc = min(G, NSUP - TT)
                zt_c = ztp.tile([81, Gc * KE * P], dt.float16, tag="ztc")
                nc.sync.dma_start(
                    out=zt_c, in_=zt[:, TT * KE * P:(TT + Gc) * KE * P])
                xct_c = xctp.tile([65, Gc * SUP], dt.float16, tag="xctc")
                nc.sync.dma_start(
                    out=xct_c, in_=xct[:, TT * SUP:(TT + Gc) * SUP])
                s4_c = sp.tile([P, Gc * NW * P], dt.float8e4, tag="s4c")
                nc.sync.dma_start(
                    out=s4_c,
                    in_=s4d[:, TT * NW * P:(TT + Gc) * NW * P])
                o_buf = op.tile([P, Gc * SUP], dt.float16, tag="obuf")

                # --- overflow h0 for the whole chunk, grouped per bank ---
                OV = KE - NW
                per = max(1, (SUP // HID) // OV)  # supertiles per psum bank
                h_sbO = hp.tile([P, Gc * OV * HID], dt.float16, tag="hsbo")
                for base in range(0, Gc, per):
                    cnt = min(per, Gc - base)
                    h_pso = hops.tile([P, cnt * OV * HID], mybir.dt.float32,
                                      tag="hpso")
                    for g2 in range(cnt):
                        zto = zt_c[:, (base + g2) * KE * P:]
                        for i in range(OV):
                            nc.tensor.matmul(
                                h_pso[:, (g2 * OV + i) * HID:
                                      (g2 * OV + i + 1) * HID],
                                zto[:, (NW + i) * P:(NW + i + 1) * P],
                                w_ih_t, start=True, stop=True)
                    dst_sl = h_sbO[:, base * OV * HID:(base + cnt) * OV * HID]
                    if apply_bh:
                        relu_h(dst_sl, h_pso, ntile=cnt * OV)
                    else:
                        nc.vector.tensor_scalar_max(dst_sl, h_pso, 0.0)

                for g in range(Gc):
                    T = TT + g
                    ztg = zt_c[:, g * KE * P:(g + 1) * KE * P]
                    xctg = xct_c[:, g * SUP:(g + 1) * SUP]

                    # --- h0: 4 window tiles in one bank ---
                    h_sb = hp.tile([P, NW * HID], dt.float16)
                    h_ps4 = hps.tile([P, NW * HID], mybir.dt.float32)
                    for jj in range(NW):
                        nc.tensor.matmul(h_ps4[:, jj * HID:(jj + 1) * HID],
                                         ztg[:, jj * P:(jj + 1) * P],
                                         w_ih_t, start=True, stop=True)
                    relu_h(h_sb, h_ps4)

                    # --- one-hot scatter matrices (windows prebuilt fp8) ---
                    j0 = T * KE
                    s4 = s4_c[:, g * NW * P:(g + 1) * NW * P].rearrange(
                        "p (w q) -> p w q", w=NW)
                    s_o = []
                    for oo in range(NW, KE):
                        s_t = sop.tile([P, SUP], dt.float16)
                        nc.vector.tensor_scalar(
                            s_t, iota_t, ldrel_t[:, j0 + oo:j0 + oo + 1],
                            None, op0=EQ)
                        s_o.append(s_t)

                    # --- scatter-add on the PE: m_T[h, n] += h^T @ S ---
                    # first writer covers the full bank width (start=True),
                    # everything after accumulates (start=False)
                    m_psum = mps.tile([P, SUP], mybir.dt.float32)
                    for i in range(KE - NW):
                        nc.tensor.matmul(m_psum,
                                         h_sbO[:, (g * OV + i) * HID:
                                               (g * OV + i + 1) * HID],
                                         s_o[i], start=(i == 0), stop=False,
                                         skip_group_check=True)
                    for jj in range(NW):
                        nc.tensor.matmul(m_psum[:, jj * P:(jj + 1) * P],
                                         h_sb[:, jj * HID:(jj + 1) * HID],
                                         s4[:, jj, :], start=False,
                                         stop=(jj == NW - 1),
                                         skip_group_check=True)

                    m_t = mp.tile([P, SUP], dt.float16)
                    nc.vector.tensor_copy(m_t, m_psum)

                    # --- out^T[o, v] = relu(W_o1^T @ xct + W_o2^T @ m_T) ---
                    o_psum = ops.tile([P, SUP], mybir.dt.float32)
                    nc.tensor.matmul(o_psum, w_o1_t, xctg,
                                     start=True, stop=False)
                    nc.tensor.matmul(o_psum, w_o2_t, m_t,
                                     start=False, stop=True)
                    nc.scalar.activation(o_buf[:, g * SUP:(g + 1) * SUP],
                                         o_psum, RELU)

                nc.sync.dma_start(out=outT[:, TT * SUP:(TT + Gc) * SUP],
                                  in_=o_buf)

    nc.compile()
    return nc


def kernel(**inputs):
    x = np.ascontiguousarray(np.asarray(inputs["x"]), dtype=F32)
    edge_index = np.asarray(inputs["edge_index"]).astype(np.int64)
    edge_attr = np.ascontiguousarray(np.asarray(inputs["edge_attr"]), dtype=F32)
    rev = np.asarray(inputs["rev_edge_index"]).astype(np.int64)
    W_i = np.asarray(inputs["W_i"], dtype=F32)
    b_i = np.asarray(inputs["b_i"], dtype=F32)
    W_h = np.asarray(inputs["W_h"], dtype=F32)
    b_h = np.asarray(inputs["b_h"], dtype=F32)
    W_o = np.asarray(inputs["W_o"], dtype=F32)
    b_o = np.asarray(inputs["b_o"], dtype=F32)

    src, dst = edge_index[0], edge_index[1]

    if not _check_fast_path_ok(src, dst, rev, x, edge_attr,
                               W_i, b_i, W_h, b_h, W_o, b_o):
        return _reference_fallback(x, edge_index, edge_attr, rev,
                                   W_i, b_i, W_h, b_h, W_o, b_o)

    from concourse.bass_utils import run_bass_kernel_spmd

    # ---- host-side graph partition / sort (indices only) ----
    order = np.argsort(dst, kind="stable")   # global sort by dst
    dst_s = dst[order]
    # per-core contiguous ranges in the sorted edge list
    core_starts = np.searchsorted(dst_s, np.arange(0, N + NL, NL))

    # Edge-to-slot assignment: supertiles of SUP=512 local nodes, split in 4
    # fixed 128-node windows. The first 128 edges of window jj go to edge
    # tile jj (one-hot columns relative to the window); the rest overflow
    # into full-width tiles 4..KE-1.
    NW = 4
    per_core = []
    KE = 5
    for c in range(NCORES):
        e0, e1 = core_starts[c], core_starts[c + 1]
        ne = e1 - e0
        ld = dst_s[e0:e1] - c * NL           # local dst in [0, NL), sorted
        gidx = ld // P                       # 128-node window id (0..391)
        gstarts = np.searchsorted(ld, np.arange(0, NPAD + P, P))
        r = np.arange(ne) - gstarts[gidx]    # rank within window
        wmask = r < P
        T = ld // SUP                        # supertile id
        tstarts = np.searchsorted(ld, np.arange(0, NPAD + SUP, SUP))
        ocum = np.cumsum(~wmask)             # overflow count up to (incl) i
        prefix = np.concatenate(([0], ocum))
        o_rank = (ocum - 1) - prefix[tstarts[T]]
        n_ov = int(np.sum(~wmask))
        max_ov = int(o_rank[~wmask].max()) + 1 if n_ov else 0
        KE = max(KE, NW + max(1, int(np.ceil(max_ov / P))))
        per_core.append((e0, e1, ld, gidx, r, wmask, T, o_rank))

    # Extremely skewed degree distributions would blow up the overflow
    # capacity (KE); the documented generator (uniform dst) keeps KE at 5.
    if KE > 8:
        return _reference_fallback(x, edge_index, edge_attr, rev,
                                   W_i, b_i, W_h, b_h, W_o, b_o)

    apply_bh = bool(np.any(b_h))
    EPAD = NSUP * KE * P

    # ---- shared constant tensors ----
    w_ih_np = np.concatenate([W_i, b_i[None, :]], axis=0).astype(F16)      # [81,128]
    w_o1_np = np.concatenate([W_o[:XD], b_o[None, :]], axis=0).astype(F16)  # [65,128]
    w_o2_np = np.ascontiguousarray(W_o[XD:]).astype(F16)                   # [128,128]
    iota_np = np.broadcast_to(np.arange(SUP, dtype=F16), (P, SUP)).copy()
    bh_np = np.broadcast_to(b_h.astype(F16), (P, HID)).copy()

    x16t = np.ascontiguousarray(x.T.astype(F16))            # [64, N]
    ea16t = np.ascontiguousarray(edge_attr.T.astype(F16))   # [16, E]

    in_maps = []
    for c in range(NCORES):
        e0, e1, ld, gidx, r, wmask, T, o_rank = per_core[c]
        eids = order[e0:e1]

        # slot per edge: window edges -> tile (gidx%4) of supertile T at
        # rank r; overflow edges -> tiles NW.. at rank o_rank.
        base = T * (KE * P)
        slots = np.where(
            wmask,
            base + (gidx % NW) * P + r,
            base + NW * P + o_rank,
        )
        # one-hot column value (window tiles are window-relative)
        ldrel_val = np.where(wmask, ld % P, ld % SUP).astype(F32)

        zt_np = np.zeros((81, EPAD), dtype=F16)
        zt_np[0:XD, slots] = x16t[:, src[eids]]
        zt_np[XD:XD + EAD, slots] = ea16t[:, eids]
        zt_np[80, slots] = 1.0

        ldrel_np = np.full((P, NSUP * KE), -1.0, dtype=F32)
        ldrel_np[slots % P, slots // P] = ldrel_val

        s4_np = np.zeros((P, NSUP * NW * P), dtype=ml_dtypes.float8_e4m3)
        ws = wmask
        s4_np[r[ws], T[ws] * (NW * P) + (gidx[ws] % NW) * P + (ld[ws] % P)] = 1.0

        xct_np = np.zeros((65, NPAD), dtype=F16)
        xct_np[0:XD, :NL] = x16t[:, c * NL:(c + 1) * NL]
        xct_np[64, :] = 1.0

        in_maps.append({
            "zt": zt_np, "xct": xct_np, "ldrel": ldrel_np,
            "s4d": s4_np, "iota": iota_np,
            "w_ih": w_ih_np, "w_o1": w_o1_np, "w_o2": w_o2_np, "bh_b": bh_np,
        })

    key = (KE, apply_bh)
    if key not in _PROGRAM_CACHE:
        _PROGRAM_CACHE[key] = _build_program(KE, apply_bh)
    nc = _PROGRAM_CACHE[key]

    import os
    trace = bool(os.environ.get("BMP_TRACE"))
    res = run_bass_kernel_spmd(nc, in_maps, core_ids=list(range(NCORES)),
                               trace=trace)
    if trace:
        global LAST_EXEC_TIME_NS, LAST_TRACE
        LAST_EXEC_TIME_NS = res.exec_time_ns
        LAST_TRACE = res.instructions_and_trace
    out = np.empty((N, HID), dtype=F32)
    for c in range(NCORES):
        out[c * NL:(c + 1) * NL] = res.results[c]["outT"][:, :NL].T.astype(F32)
    return out


# revision 29
# speedup vs baseline: 1.0355x; 1.0241x over previous
"""Trainium2 Bass kernel for nn_BondMessagePassing (D-MPNN style GNN).

Contract: kernel(**inputs) takes FULL unsharded inputs (as produced by the
reference's setup_inputs) and returns the FULL output [400000, 128] float32.

Math: the reference builds edges in exact reverse pairs (edge 2k+1 is the
reverse of edge 2k, rev_edge_index = i^1), which makes dst[rev] == src.
Consequently the two scatter-adds inside every message-passing iteration
cancel exactly (same multiset of h-rows lands at each node with opposite
sign), so m == 0 in exact arithmetic and h stays at relu(h0 + b_h) for the
whole loop. The output reduces to

    h   = relu(relu([x[src], edge_attr] @ W_i + b_i) + b_h)
    m   = scatter_add(h, dst)            # one scatter, by destination node
    out = relu([x, m] @ W_o + b_o)

This identity is verified at runtime from the actual index tensors (rev is a
bijection with dst[rev] == src); if it does not hold, a numpy fallback
reproduces the reference loop exactly.

Sharding: nodes are range-partitioned across the 8 cores (50000 nodes each);
each core receives exactly the edges whose dst lands in its range (sorted by
dst) so the scatter-add is core-local and the output rows are a contiguous
slice -- no collectives. The scatter-add itself is done on the TensorEngine
as a matmul against a one-hot matrix built on-device with an iota/is_equal
compare, over supertiles of 512 nodes (one PSUM bank per supertile).
"""

import ml_dtypes
import numpy as np

# ---- problem constants (hardcoded per contract) ----
N = 400000
E = 400000
XD = 64        # node feature dim
EAD = 16       # edge feature dim
HID = 128      # hidden dim
DEPTH = 5
NCORES = 8
NL = N // NCORES          # nodes per core
SUP = 512                 # nodes per supertile (one PSUM bank of fp32)
NSUP = (NL + SUP - 1) // SUP
NPAD = NSUP * SUP         # padded nodes per core
P = 128                   # partitions / tile rows

F16 = np.float16
F32 = np.float32


def _check_fast_path_ok(src, dst, rev, x, edge_attr, W_i, b_i, W_h, b_h, W_o, b_o):
    """True iff the loop-cancellation identity holds and fp16 is safe."""
    if src.shape != (E,) or dst.shape != (E,):
        return False
    if rev.shape != (E,):
        return False
    # rev must be a bijection on [0, E)
    if rev.min() < 0 or rev.max() >= E:
        return False
    seen = np.zeros(E, dtype=bool)
    seen[rev] = True
    if not seen.all():
        return False
    # the cancellation identity
    if not np.array_equal(dst[rev], src):
        return False
    if src.min() < 0 or src.max() >= N or dst.min() < 0 or dst.max() >= N:
        return False
    # fp16 magnitude safety (values stream through fp16 operands)
    mx = float(np.abs(x).max(initial=0.0))
    mea = float(np.abs(edge_attr).max(initial=0.0))
    mw = max(float(np.abs(W_i).max(initial=0.0)), float(np.abs(W_o).max(initial=0.0)))
    mb = max(float(np.abs(b_i).max(initial=0.0)), float(np.abs(b_h).max(initial=0.0)),
             float(np.abs(b_o).max(initial=0.0)))
    hbound = 81.0 * max(mx, mea, 1.0) * max(mw, 1.0) + mb
    if not np.isfinite(hbound) or hbound > 2.0e4:
        return False
    return True


def _reference_fallback(x, edge_index, edge_attr, rev_edge_index,
                        W_i, b_i, W_h, b_h, W_o, b_o):
    """Exact numpy replication of the reference (used only if the structural
    identity does not hold, which cannot happen for the documented input
    generator)."""
    def san(t):
        return np.nan_to_num(t, nan=0.0, posinf=1000.0, neginf=-1000.0)

    src, dst = edge_index[0], edge_index[1]
    h0 = np.maximum(
        np.concatenate([x[src], edge_attr], axis=1) @ W_i + b_i, 0.0
    ).astype(F32)
    h = h0
    for _ in range(1, DEPTH):
        m = np.zeros_like(h)
        np.add.at(m, dst, h)
        np.add.at(m, src, -h[rev_edge_index])
        m = san(m) @ W_h + b_h
        h = np.maximum(h0 + m, 0.0).astype(F32)
    m_final = np.zeros_like(h)
    np.add.at(m_final, dst, h)
    h_cat = np.concatenate([x, san(m_final)], axis=1)
    out = np.maximum(h_cat @ W_o + b_o, 0.0).astype(F32)
    return san(out)


_PROGRAM_CACHE = {}


def _build_program(KE, apply_bh):
    """Build the (SPMD-identical) Bass program.

    KE = edge tiles per supertile: tiles 0..3 hold edges of the fixed
    128-node sub-windows (one-hot column range 128*jj..128*jj+127), tiles
    4..KE-1 are full-width overflow tiles. The layout (which edge goes to
    which tile) is data-dependent, but the program structure is not, so one
    program serves all 8 cores (SPMD).
    """
    import concourse.bacc as bacc
    import concourse.mybir as mybir
    import concourse.tile as tile

    nc = bacc.Bacc("TRN2", target_bir_lowering=False, debug=False,
                   num_devices=NCORES)
    dt = mybir.dt
    EPAD = NSUP * KE * P
    G = 7  # supertiles per DMA chunk

    zt = nc.dram_tensor("zt", [81, EPAD], dt.float16, kind="ExternalInput")
    xct = nc.dram_tensor("xct", [65, NPAD], dt.float16, kind="ExternalInput")
    ldrel = nc.dram_tensor("ldrel", [P, NSUP * KE], dt.float32,
                           kind="ExternalInput")
    s4d = nc.dram_tensor("s4d", [P, NSUP * 4 * P], dt.float8e4,
                         kind="ExternalInput")
    iota = nc.dram_tensor("iota", [P, SUP], dt.float16, kind="ExternalInput")
    w_ih = nc.dram_tensor("w_ih", [81, HID], dt.float16, kind="ExternalInput")
    w_o1 = nc.dram_tensor("w_o1", [65, HID], dt.float16, kind="ExternalInput")
    w_o2 = nc.dram_tensor("w_o2", [HID, HID], dt.float16, kind="ExternalInput")
    bh_b = nc.dram_tensor("bh_b", [P, HID], dt.float16, kind="ExternalInput")
    # output is produced TRANSPOSED ([hidden, node]); host transposes back
    outT = nc.dram_tensor("outT", [HID, NPAD], dt.float16,
                          kind="ExternalOutput")

    RELU = mybir.ActivationFunctionType.Relu
    EQ = mybir.AluOpType.is_equal
    NW = 4  # number of fixed-window tiles per supertile

    with tile.TileContext(nc) as tc:
        with (
            tc.tile_pool(name="consts", bufs=1) as consts,
            tc.tile_pool(name="ztp", bufs=3) as ztp,
            tc.tile_pool(name="xctp", bufs=3) as xctp,
            tc.tile_pool(name="hp", bufs=4) as hp,
            tc.tile_pool(name="sp", bufs=3) as sp,
            tc.tile_pool(name="sop", bufs=3) as sop,
            tc.tile_pool(name="mp", bufs=4) as mp,
            tc.tile_pool(name="op", bufs=3) as op,
            tc.tile_pool(name="hps", bufs=2, space="PSUM") as hps,
            tc.tile_pool(name="hops", bufs=1, space="PSUM") as hops,
            tc.tile_pool(name="mps", bufs=3, space="PSUM") as mps,
            tc.tile_pool(name="ops", bufs=2, space="PSUM") as ops,
        ):
            w_ih_t = consts.tile([81, HID], dt.float16)
            nc.sync.dma_start(out=w_ih_t, in_=w_ih[:, :])
            w_o1_t = consts.tile([65, HID], dt.float16)
            nc.sync.dma_start(out=w_o1_t, in_=w_o1[:, :])
            w_o2_t = consts.tile([HID, HID], dt.float16)
            nc.sync.dma_start(out=w_o2_t, in_=w_o2[:, :])
            iota_t = consts.tile([P, SUP], dt.float16)
            nc.sync.dma_start(out=iota_t, in_=iota[:, :])
            ldrel_t = consts.tile([P, NSUP * KE], dt.float32)
            nc.sync.dma_start(out=ldrel_t, in_=ldrel[:, :])
            if apply_bh:
                bh_t = consts.tile([P, HID], dt.float16)
                nc.sync.dma_start(out=bh_t, in_=bh_b[:, :])

            def relu_h(h_sbuf, h_psum, ntile=NW):
                nc.scalar.activation(h_sbuf, h_psum, RELU)
                if apply_bh:
                    for q in range(ntile):
                        nc.vector.tensor_tensor(
                            h_sbuf[:, q * HID:(q + 1) * HID],
                            h_sbuf[:, q * HID:(q + 1) * HID], bh_t,
                            op=mybir.AluOpType.add)
                    nc.scalar.activation(h_sbuf, h_sbuf, RELU)

            sched = [(t, G) for t in range(0, NSUP - 14, G)]
            t0 = len(sched) * G
            for g in (7, 4, 2, 1):
                sched.append((t0, g)); t0 += g
            assert t0 == NSUP
            for TT, Gc in sched:
                zt_c = ztp.tile([81, Gc * KE * P], dt.float16, tag="ztc")
                nc.sync.dma_start(
                    out=zt_c, in_=zt[:, TT * KE * P:(TT + Gc) * KE * P])
                xct_c = xctp.tile([65, Gc * SUP], dt.float16, tag="xctc")
                nc.sync.dma_start(
                    out=xct_c, in_=xct[:, TT * SUP:(TT + Gc) * SUP])
                s4_c = sp.tile([P, Gc * NW * P], dt.float8e4, tag="s4c")
                nc.sync.dma_start(
                    out=s4_c,
                    in_=s4d[:, TT * NW * P:(TT + Gc) * NW * P])
                o_buf = op.tile([P, Gc * SUP], dt.float16, tag="obuf")

                # --- overflow h0 for the whole chunk, grouped per bank ---
                OV = KE - NW
                per = max(1, (SUP // HID) // OV)  # supertiles per psum bank
                h_sbO = hp.tile([P, Gc * OV * HID], dt.float16, tag="hsbo")
                for base in range(0, Gc, per):
                    cnt = min(per, Gc - base)
                    h_pso = hops.tile([P, cnt * OV * HID], mybir.dt.float32,
                                      tag="hpso")
                    for g2 in range(cnt):
                        zto = zt_c[:, (base + g2) * KE * P:]
                        for i in range(OV):
                            nc.tensor.matmul(
                                h_pso[:, (g2 * OV + i) * HID:
                                      (g2 * OV + i + 1) * HID],
                                zto[:, (NW + i) * P:(NW + i + 1) * P],
                                w_ih_t, start=True, stop=True)
                    dst_sl = h_sbO[:, base * OV * HID:(base + cnt) * OV * HID]
                    if apply_bh:
                        relu_h(dst_sl, h_pso, ntile=cnt * OV)
                    else:
                        nc.vector.tensor_scalar_max(dst_sl, h_pso, 0.0)

                for g in range(Gc):
                    T = TT + g
                    ztg = zt_c[:, g * KE * P:(g + 1) * KE * P]
                    xctg = xct_c[:, g * SUP:(g + 1) * SUP]

                    # --- h0: 4 window tiles in one bank ---
                    h_sb = hp.tile([P, NW * HID], dt.float16)
                    h_ps4 = hps.tile([P, NW * HID], mybir.dt.float32)
                    for jj in range(NW):
                        nc.tensor.matmul(h_ps4[:, jj * HID:(jj + 1) * HID],
                                         ztg[:, jj * P:(jj + 1) * P],
                                         w_ih_t, start=True, stop=True)
                    relu_h(h_sb, h_ps4)

                    # --- one-hot scatter matrices (windows prebuilt fp8) ---
                    j0 = T * KE
                    s4 = s4_c[:, g * NW * P:(g + 1) * NW * P].rearrange(
                        "p (w q) -> p w q", w=NW)
                    s_o = []
                    for oo in range(NW, KE):
                        s_t = sop.tile([P, SUP], dt.float16)
                        nc.vector.tensor_scalar(
                            s_t, iota_t, ldrel_t[:, j0 + oo:j0 + oo + 1],
                            None, op0=EQ)
                        s_o.append(s_t)

                    # --- scatter-add on the PE: m_T[h, n] += h^T @ S ---
                    # first writer covers the full bank width (start=True),
                    # everything after accumulates (start=False)
                    m_psum = mps.tile([P, SUP], mybir.dt.float32)
                    for i in range(KE - NW):
                        nc.tensor.matmul(m_psum,
                                         h_sbO[:, (g * OV + i) * HID:
                                               (g * OV + i + 1) * HID],
                                         s_o[i], start=(i == 0), stop=False,
                                         skip_group_check=True)
                    for jj in range(NW):
                        nc.tensor.matmul(m_psum[:, jj * P:(jj + 1) * P],
                                         h_sb[:, jj * HID:(jj + 1) * HID],
                                         s4[:, jj, :], start=False,
                                         stop=(jj == NW - 1),
                                         skip_group_check=True)

                    m_t = mp.tile([P, SUP], dt.float16)
                    nc.vector.tensor_copy(m_t, m_psum)

                    # --- out^T[o, v] = relu(W_o1^T @ xct + W_o2^T @ m_T) ---
                    o_psum = ops.tile([P, SUP], mybir.dt.float32)
                    nc.tensor.matmul(o_psum, w_o1_t, xctg,
                                     start=True, stop=False)
                    nc.tensor.matmul(o_psum, w_o2_t, m_t,
                                     start=False, stop=True)
                    nc.scalar.activation(o_buf[:, g * SUP:(g + 1) * SUP],
                                         o_psum, RELU)

                nc.sync.dma_start(out=outT[:, TT * SUP:(TT + Gc) * SUP],
                                  in_=o_buf)

    nc.compile()
    return nc


def kernel(**inputs):
    x = np.ascontiguousarray(np.asarray(inputs["x"]), dtype=F32)
    edge_index = np.asarray(inputs["edge_index"]).astype(np.int64)
    edge_attr = np.ascontiguousarray(np.asarray(inputs["edge_attr"]), dtype=F32)
    rev = np.asarray(inputs["rev_edge_index"]).astype(np.int64)
    W_i = np.asarray(inputs["W_i"], dtype=F32)
    b_i = np.asarray(inputs["b_i"], dtype=F32)
    W_h = np.asarray(inputs["W_h"], dtype=F32)
    b_h = np.asarray(inputs["b_h"], dtype=F32)
    W_o = np.asarray(inputs["W_o"], dtype=F32)
    b_o = np.asarray(inputs["b_o"], dtype=F32)

    src, dst = edge_index[0], edge_index[1]

    if not _check_fast_path_ok(src, dst, rev, x, edge_attr,
                               W_i, b_i, W_h, b_h, W_o, b_o):
        return _reference_fallback(x, edge_index, edge_attr, rev,
                                   W_i, b_i, W_h, b_h, W_o, b_o)

    from concourse.bass_utils import run_bass_kernel_spmd

    # ---- host-side graph partition / sort (indices only) ----
    order = np.argsort(dst, kind="stable")   # global sort by dst
    dst_s = dst[order]
    # per-core contiguous ranges in the sorted edge list
    core_starts = np.searchsorted(dst_s, np.arange(0, N + NL, NL))

    # Edge-to-slot assignment: supertiles of SUP=512 local nodes, split in 4
    # fixed 128-node windows. The first 128 edges of window jj go to edge
    # tile jj (one-hot columns relative to the window); the rest overflow
    # into full-width tiles 4..KE-1.
    NW = 4
    per_core = []
    KE = 5
    for c in range(NCORES):
        e0, e1 = core_starts[c], core_starts[c + 1]
        ne = e1 - e0
        ld = dst_s[e0:e1] - c * NL           # local dst in [0, NL), sorted
        gidx = ld // P                       # 128-node window id (0..391)
        gstarts = np.searchsorted(ld, np.arange(0, NPAD + P, P))
        r = np.arange(ne) - gstarts[gidx]    # rank within window
        wmask = r < P
        T = ld // SUP                        # supertile id
        tstarts = np.searchsorted(ld, np.arange(0, NPAD + SUP, SUP))
        ocum = np.cumsum(~wmask)             # overflow count up to (incl) i
        prefix = np.concatenate(([0], ocum))
        o_rank = (ocum - 1) - prefix[tstarts[T]]
        n_ov = int(np.sum(~wmask))
        max_ov = int(o_rank[~wmask].max()) + 1 if n_ov else 0
        KE = max(KE, NW + max(1, int(np.ceil(max_ov / P))))
        per_core.append((e0, e1, ld, gidx, r, wmask, T, o_rank))

    # Extremely skewed degree distributions would blow up the overflow
    # capacity (KE); the documented generator (uniform dst) keeps KE at 5.
    if KE > 8:
        return _reference_fallback(x, edge_index, edge_attr, rev,
                                   W_i, b_i, W_h, b_h, W_o, b_o)

    apply_bh = bool(np.any(b_h))
    EPAD = NSUP * KE * P

    # ---- shared constant tensors ----
    w_ih_np = np.concatenate([W_i, b_i[None, :]], axis=0).astype(F16)      # [81,128]
    w_o1_np = np.concatenate([W_o[:XD], b_o[None, :]], axis=0).astype(F16)  # [65,128]
    w_o2_np = np.ascontiguousarray(W_o[XD:]).astype(F16)                   # [128,128]
    iota_np = np.broadcast_to(np.arange(SUP, dtype=F16), (P, SUP)).copy()
    bh_np = np.broadcast_to(b_h.astype(F16), (P, HID)).copy()

    x16t = np.ascontiguousarray(x.T.astype(F16))            # [64, N]
    ea16t = np.ascontiguousarray(edge_attr.T.astype(F16))   # [16, E]

    in_maps = []
    for c in range(NCORES):
        e0, e1, ld, gidx, r, wmask, T, o_rank = per_core[c]
        eids = order[e0:e1]

        # slot per edge: window edges -> tile (gidx%4) of supertile T at
        # rank r; overflow edges -> tiles NW.. at rank o_rank.
        base = T * (KE * P)
        slots = np.where(
            wmask,
            base + (gidx % NW) * P + r,
            base + NW * P + o_rank,
        )
        # one-hot column value (window tiles are window-relative)
        ldrel_val = np.where(wmask, ld % P, ld % SUP).astype(F32)

        zt_np = np.zeros((81, EPAD), dtype=F16)
        zt_np[0:XD, slots] = x16t[:, src[eids]]
        zt_np[XD:XD + EAD, slots] = ea16t[:, eids]
        zt_np[80, slots] = 1.0

        ldrel_np = np.full((P, NSUP * KE), -1.0, dtype=F32)
        ldrel_np[slots % P, slots // P] = ldrel_val

        s4_np = np.zeros((P, NSUP * NW * P), dtype=ml_dtypes.float8_e4m3)
        ws = wmask
        s4_np[r[ws], T[ws] * (NW * P) + (gidx[ws] % NW) * P + (ld[ws] % P)] = 1.0

        xct_np = np.zeros((65, NPAD), dtype=F16)
        xct_np[0:XD, :NL] = x16t[:, c * NL:(c + 1) * NL]
        xct_np[64, :] = 1.0

        in_maps.append({
            "zt": zt_np, "xct": xct_np, "ldrel": ldrel_np,
            "s4d": s4_np, "iota": iota_np,
            "w_ih": w_ih_np, "w_o1": w_o1_np, "w_o2": w_o2_np, "bh_b": bh_np,
        })

    key = (KE, apply_bh)
    if key not in _PROGRAM_CACHE:
        _PROGRAM_CACHE[key] = _build_program(KE, apply_bh)
    nc = _PROGRAM_CACHE[key]

    import os
    trace = bool(os.environ.get("BMP_TRACE"))
    res = run_bass_kernel_spmd(nc, in_maps, core_ids=list(range(NCORES)),
                               trace=trace)
    if trace:
        global LAST_EXEC_TIME_NS, LAST_TRACE
        LAST_EXEC_TIME_NS = res.exec_time_ns
        LAST_TRACE = res.instructions_and_trace
    out = np.empty((N, HID), dtype=F32)
    for c in range(NCORES):
        out[c * NL:(c + 1) * NL] = res.results[c]["outT"][:, :NL].T.astype(F32)
    return out
